# revision 34
# baseline (speedup 1.0000x reference)
"""BatchedACE (soft clustered linear attention) Trainium2 kernel.

Full inputs -> full output. Sharding: N = M*B*H batch axis across 8 cores;
core c handles (m, b) = (c//2, c%2), i.e. all 8 heads of one (ensemble,
batch) pair. Host pre-work per core: K/Q transposed to d-major (q=[d|d]
even/odd head pairs, p, T) and converted to bf16; V gets a ones column
baked in (for the A row-sum) and is bf16.

Phase 1 (software-pipelined, skew 2: front(i) runs with tail(i-2)):
  front: DMA (prefetch +1, sync HWDGE queue); projKQ = planes^T @ [q|k]T
    (Q rows 0-63, K rows 64-127 of one PSUM bank via tile_position);
    tanhKQ (1 ACT op); logitsQT = blockdiag^T @ tanhQ (2 mm, head order
    0,2,4,6,1,3,5,7); expQT; logitsK via paired bd_eo (4 mm, base-64
    operands, natural head order); expK.
  tail:  gsum = ind^T @ expQT (2 mm); recipQS (DVE) -> bf16 cast;
    probsK = expK * recipK (group softmax16: DVE reduce + GPS multiply);
    bcast = indT^T @ recipQS (2 mm); stash probsQT bf16 (DVE);
    b_sum/A accumulate in 2 persistent PSUM banks via probsK^T @ [V|1].
  PSUM pool rotation order (gsum, proj, logitsQT, logitsK, recipQb) is
  chosen so no buffer reuse waits on a chain-end op.

Phase 2: E = b_sum/(A+eps) (ACT scale); out = stash^T @ E per (h, tile);
  bf16 output, row-interleaved DMA split across sync+scalar HWDGE queues
  (1KB packets spread over all 16 DMA engines). Host transposes
  (T,H,D)->(H,T,D) and casts back to f32.

Measured: ~232 us on-device (from 739 us baseline), rel err 9.6e-3.
"""

import itertools

import numpy as np
import ml_dtypes

import concourse.bacc as bacc
import concourse.mybir as mybir
import concourse.tile as tile

F32 = mybir.dt.float32
BF16 = mybir.dt.bfloat16
AF = mybir.ActivationFunctionType
MULT = mybir.AluOpType.mult

D_K, K_BITS, L_TAB, M_ENS = 64, 4, 8, 4
R = 1 << K_BITS          # 16
S = L_TAB * R            # 128
B, T, H = 2, 4096, 8
EPS = 1e-06
HD = H * D_K             # 512
TT = 128                 # T tile rows
NT = T // TT             # 32 tiles

# batched logitsQT column-block j holds head HEAD_AT[j]; POS inverts it
HEAD_AT = [0, 2, 4, 6, 1, 3, 5, 7]
POS = [HEAD_AT.index(h) for h in range(H)]


def _build_module():
    nc = bacc.Bacc("TRN2", target_bir_lowering=False, debug=False,
                   num_devices=8, enable_asserts=False)

    KT = nc.dram_tensor("KT", [128, 4, T], BF16, kind="ExternalInput")
    QT = nc.dram_tensor("QT", [128, 4, T], BF16, kind="ExternalInput")
    V = nc.dram_tensor("V", [T, H, 65], BF16, kind="ExternalInput")
    planes_both = nc.dram_tensor("planes_both", [128, 64], BF16, kind="ExternalInput")
    bd_eo2 = nc.dram_tensor("bd_eo2", [128, 256], BF16, kind="ExternalInput")
    blockdiag4 = nc.dram_tensor("blockdiag4", [128, 128], BF16, kind="ExternalInput")
    ind = nc.dram_tensor("ind", [128, 8], BF16, kind="ExternalInput")
    indT = nc.dram_tensor("indT", [8, 128], BF16, kind="ExternalInput")
    O = nc.dram_tensor("O", [T, HD], BF16, kind="ExternalOutput")

    with tile.TileContext(nc) as tc:
        with (
            tc.tile_pool(name="pconst", bufs=1) as pconst,
            tc.tile_pool(name="pstash", bufs=1) as pstash,
            tc.tile_pool(name="pin", bufs=5) as pin,
            tc.tile_pool(name="pmid", bufs=3) as pmid,
            tc.tile_pool(name="pout", bufs=3) as pout,
            tc.tile_pool(name="psmall", bufs=4) as psmall,
            tc.tile_pool(name="pacc", bufs=1, space="PSUM") as pacc,
            tc.tile_pool(name="pwork", bufs=3, space="PSUM") as pwork,
        ):
            # ---- first input tiles before consts (sync queue is idle)
            def issue_dma(ti):
                rows = slice(ti * TT, (ti + 1) * TT)
                kT_sb = pin.tile([128, 4, TT], BF16, tag="ktsb")
                nc.sync.dma_start(kT_sb[:], KT[:, :, rows])
                qT_sb = pin.tile([128, 4, TT], BF16, tag="qtsb")
                nc.sync.dma_start(qT_sb[:], QT[:, :, rows])
                tV = pin.tile([128, H, 65], BF16, tag="tv")
                nc.sync.dma_start(tV[:], V[rows, :, :])
                return {"kT": kT_sb, "qT": qT_sb, "tV": tV}

            stages = {}
            stages[0] = issue_dma(0)
            stages[1] = issue_dma(1)

            # ---- constants to SBUF
            planes_both_sb = pconst.tile([128, 64], BF16)
            nc.scalar.dma_start(planes_both_sb[:], planes_both[:])
            bd_eo2_sb = pconst.tile([128, 256], BF16)
            nc.scalar.dma_start(bd_eo2_sb[:], bd_eo2[:])
            blockdiag4_sb = pconst.tile([128, 128], BF16)
            nc.scalar.dma_start(blockdiag4_sb[:], blockdiag4[:])
            ind_sb = pconst.tile([128, 8], BF16)
            nc.scalar.dma_start(ind_sb[:], ind[:])
            indT_sb = pconst.tile([8, 128], BF16)
            nc.scalar.dma_start(indT_sb[:], indT[:])
            zrow = pconst.tile([1, 512], F32)
            nc.gpsimd.memset(zrow[:], 0.0)
            zcol = pconst.tile([1, 128], F32)
            nc.gpsimd.memset(zcol[:], 0.0)

            # ---- persistent PSUM accumulators: 4 heads per bank
            accA = pacc.tile([128, 4 * 65], F32)
            accB = pacc.tile([128, 4 * 65], F32)
            # zero-fill via a K=1 matmul of zeros: sets has_written for the
            # whole bank so every real b_sum matmul can accumulate
            # (start=False) in any order.
            nc.tensor.matmul(accA[:, 0:260], zcol[:], zrow[:, 0:260],
                             start=True, stop=False, skip_group_check=True)
            nc.tensor.matmul(accB[:, 0:260], zcol[:], zrow[:, 0:260],
                             start=True, stop=False, skip_group_check=True)

            # probsQ^T stash: (s, block j, tile, t) bf16
            stash = pstash.tile([128, H, NT, TT], BF16)

            # ================= phase 1 (software-pipelined) =================
            # front(i): DMA(prefetched), projKQ (Q rows 0-63, K rows 64-127),
            #   tanhKQ, logitsK (base-64 operands), logitsQT, expK, expQT
            # tail(i-1): gsum, recipQS, cast(GPS), bcast, stashTT,
            #   reduceK, recipK, probsK(GPS), b_sum
            def front(ti, S):
                projKQ = pwork.tile([128, 512], F32, tag="work")
                nc.tensor.matmul(projKQ[0:64, :], planes_both_sb[:],
                                 S["qT"][:].rearrange("q p t -> q (p t)"),
                                 start=True, stop=True)
                nc.tensor.matmul(projKQ[64:128, :], planes_both_sb[:],
                                 S["kT"][:].rearrange("q p t -> q (p t)"),
                                 start=True, stop=True)
                tanhKQ = pmid.tile([128, 512], BF16, tag="thkq")
                nc.scalar.activation(tanhKQ[:], projKQ[:], AF.Tanh)

                logitsQT = pwork.tile([128, 1024], F32, tag="work")
                nc.tensor.matmul(logitsQT[:, 0:512], blockdiag4_sb[0:32, :],
                                 tanhKQ[0:32, :], start=True, stop=True)
                nc.tensor.matmul(logitsQT[:, 512:1024],
                                 blockdiag4_sb[32:64, :],
                                 tanhKQ[32:64, :], start=True, stop=True)
                expQT = pmid.tile([128, 1024], BF16, tag="expq")
                nc.scalar.activation(expQT[:], logitsQT[:], AF.Exp)

                logitsK = pwork.tile([128, 1024], F32, tag="work")
                for p in range(4):
                    nc.tensor.matmul(
                        logitsK[:, p * 256:(p + 1) * 256],
                        tanhKQ[64:128, p * 128:(p + 1) * 128],
                        bd_eo2_sb[64:128, :], start=True, stop=True)
                expK = pmid.tile([128, 1024], BF16, tag="expk")
                nc.scalar.activation(expK[:], logitsK[:], AF.Exp)
                S["expK"] = expK
                S["expQT"] = expQT

            def tail_gsum(ti, S):
                gsumQ = pwork.tile([8, 1024], F32, tag="work")
                nc.tensor.matmul(gsumQ[:, 0:512], ind_sb[:],
                                 S["expQT"][:, 0:512], start=True, stop=True)
                nc.tensor.matmul(gsumQ[:, 512:1024], ind_sb[:],
                                 S["expQT"][:, 512:1024], start=True,
                                 stop=True)
                recipQS = pmid.tile([8, 1024], F32, tag="rqs")
                nc.vector.reciprocal_approx_fast(recipQS[:], gsumQ[:])
                recipQSb = pmid.tile([8, 1024], BF16, tag="rqsb")
                nc.vector.tensor_copy(recipQSb[:], recipQS[:])
                S["recipQSb"] = recipQSb

            def tail_probsk(ti, S):
                expK = S["expK"]
                denomK = pmid.tile([128, 64], F32, tag="dk")
                nc.vector.reduce_sum(
                    denomK[:],
                    expK[:].rearrange("p (g r) -> p g r", g=H * L_TAB),
                    axis=mybir.AxisListType.X)
                recipK = pmid.tile([128, 64], F32, tag="rk")
                nc.vector.reciprocal_approx_fast(recipK[:], denomK[:])
                probsK = pmid.tile([128, 1024], BF16, tag="pk")
                nc.gpsimd.tensor_tensor(
                    probsK[:].rearrange("p (h l r) -> p h l r", h=H, l=L_TAB),
                    expK[:].rearrange("p (h l r) -> p h l r", h=H, l=L_TAB),
                    recipK[:].rearrange("p (h l) -> p h l", h=H)
                        .broadcast_to((128, H, L_TAB, R)),
                    op=MULT)
                S["probsK"] = probsK

            def tail_rest(ti, S):
                recipQb = pwork.tile([128, 1024], F32, tag="work")
                nc.tensor.matmul(recipQb[:, 0:512], indT_sb[:],
                                 S["recipQSb"][:, 0:512], start=True,
                                 stop=True)
                nc.tensor.matmul(recipQb[:, 512:1024], indT_sb[:],
                                 S["recipQSb"][:, 512:1024], start=True,
                                 stop=True)
                nc.vector.tensor_tensor(
                    stash[:, :, ti, :],
                    S["expQT"][:].rearrange("p (h t) -> p h t", h=H),
                    recipQb[:].rearrange("p (h t) -> p h t", h=H),
                    op=MULT)

                probsK = S["probsK"]
                for h in range(H):
                    acc = accA if h < 4 else accB
                    off = (h % 4) * 65
                    nc.tensor.matmul(
                        acc[:, off:off + 65],
                        probsK[:, h * 128:(h + 1) * 128],
                        S["tV"][:, h, :],
                        start=False, stop=(ti == NT - 1 and h % 4 == 3),
                        skip_group_check=True)

            for ti in range(NT):
                if ti + 2 < NT:
                    stages[ti + 2] = issue_dma(ti + 2)
                if ti >= 2:
                    tail_gsum(ti - 2, stages[ti - 2])
                front(ti, stages[ti])
                if ti >= 2:
                    tail_probsk(ti - 2, stages[ti - 2])
                    tail_rest(ti - 2, stages[ti - 2])
                    del stages[ti - 2]
            for ti in (NT - 2, NT - 1):
                tail_probsk(ti, stages[ti])
                tail_gsum(ti, stages[ti])
                tail_rest(ti, stages[ti])

            # ================= E = b_sum / (A + eps) =================
            e_tiles = []
            for h in range(H):
                acc = accA if h < 4 else accB
                off = (h % 4) * 65
                aeps = psmall.tile([128, 1], F32, tag="ae")
                nc.vector.tensor_scalar_add(aeps[:], acc[:, off + 64:off + 65],
                                            EPS)
                recipA = psmall.tile([128, 1], F32, tag="ra")
                nc.vector.reciprocal_approx_fast(recipA[:], aeps[:])
                e_h = pconst.tile([128, 64], BF16, name=f"e_{h}")
                nc.scalar.activation(e_h[:], acc[:, off:off + 64], AF.Copy,
                                     scale=recipA[:])
                e_tiles.append(e_h)

            # ================= phase 2: out = probsQT.T @ E =================
            for ti in range(NT):
                out2 = pwork.tile([128, 512], F32, tag="work")
                for h in range(H):
                    nc.tensor.matmul(out2[:, h * 64:(h + 1) * 64],
                                     stash[:, POS[h], ti, :], e_tiles[h][:],
                                     start=True, stop=True)
                outT = pout.tile([128, 512], BF16, tag="ot")
                nc.vector.tensor_copy(outT[:], out2[:])
                base = ti * TT
                nc.sync.dma_start(O[base:base + TT:2, :], outT[0:TT:2, :])
                nc.scalar.dma_start(O[base + 1:base + TT:2, :],
                                    outT[1:TT:2, :])

    nc.finalize()
    return nc


def _protos() -> np.ndarray:
    corners = np.array(list(itertools.product([-1.0, 1.0], repeat=K_BITS)),
                       dtype=np.float32)
    return corners.T  # (K_BITS, R)


def _consts_for(planes_m: np.ndarray, scale: float) -> dict:
    protos_s = (_protos() / scale).astype(np.float32)  # (4, 16)
    blockdiag = np.zeros((32, 128), np.float32)
    for l in range(L_TAB):
        blockdiag[l * K_BITS:(l + 1) * K_BITS, l * R:(l + 1) * R] = protos_s
    planes_both = np.zeros((128, 64), np.float32)
    planes_both[0:64, 0:32] = planes_m
    planes_both[64:128, 32:64] = planes_m
    bd_eo2 = np.zeros((128, 256), np.float32)
    bd_eo2[64:96, 0:128] = blockdiag
    bd_eo2[96:128, 128:256] = blockdiag
    blockdiag4 = np.concatenate([blockdiag] * 4, axis=0)
    ind = np.zeros((128, 8), np.float32)
    for s in range(S):
        ind[s, s // R] = 1.0
    return {
        "planes_both": planes_both.astype(ml_dtypes.bfloat16),
        "bd_eo2": bd_eo2.astype(ml_dtypes.bfloat16),
        "blockdiag4": blockdiag4.astype(ml_dtypes.bfloat16),
        "ind": ind.astype(ml_dtypes.bfloat16),
        "indT": np.ascontiguousarray(ind.T).astype(ml_dtypes.bfloat16),
    }


_NC_CACHE = None


def _get_module():
    global _NC_CACHE
    if _NC_CACHE is None:
        _NC_CACHE = _build_module()
    return _NC_CACHE


def _v_ones(v):
    out = np.ones((T, H, 65), np.float32)
    out[:, :, 0:64] = v.reshape(T, H, 64)
    return out.astype(ml_dtypes.bfloat16)


def make_in_maps(Khf, Vhf, Qhf, planes_T, logit_temp):
    Khf = np.asarray(Khf, np.float32)
    Vhf = np.asarray(Vhf, np.float32)
    Qhf = np.asarray(Qhf, np.float32)
    planes_T = np.asarray(planes_T, np.float32)
    scale = float(np.clip(np.exp(float(np.asarray(logit_temp))), 0.01, 20.0))
    in_maps = []
    for c in range(8):
        m, b = c // 2, c % 2
        consts = _consts_for(planes_T[m], scale)
        def pre_t(x):
            # (T, H*D) -> (q=[d|d], p, T): q<64 is head 2p, q>=64 head 2p+1
            x3 = x.reshape(T, 4, 2, D_K)          # (t, p, r, d)
            return np.ascontiguousarray(
                x3.transpose(2, 3, 1, 0).reshape(128, 4, T)
            ).astype(ml_dtypes.bfloat16)
        in_maps.append({
            "KT": pre_t(Khf[m, b].reshape(T, HD)),
            "QT": pre_t(Qhf[m, b].reshape(T, HD)),
            "V": _v_ones(Vhf[m, b].reshape(T, HD)),
            **consts,
        })
    return in_maps


def assemble_output(results) -> np.ndarray:
    out = np.empty((M_ENS, B, H, T, D_K), np.float32)
    for c in range(8):
        out[c // 2, c % 2] = results[c]["O"].astype(np.float32).reshape(
            T, H, D_K).transpose(1, 0, 2)
    return out


def kernel(Khf, Vhf, Qhf, planes_T, logit_temp) -> np.ndarray:
    from concourse.bass_utils import run_bass_kernel_spmd
    nc = _get_module()
    in_maps = make_in_maps(Khf, Vhf, Qhf, planes_T, logit_temp)
    res = run_bass_kernel_spmd(nc, in_maps, list(range(8)))
    return assemble_output(res.results)


# revision 35
# speedup vs baseline: 1.0421x; 1.0421x over previous
"""BatchedACE (soft clustered linear attention) Trainium2 kernel.

Full inputs -> full output. Sharding: N = M*B*H batch axis across 8 cores;
core c handles (m, b) = (c//2, c%2), i.e. all 8 heads of one (ensemble,
batch) pair. Host pre-work per core: K/Q transposed to d-major (q=[d|d]
even/odd head pairs, p, T) and converted to bf16; V gets a ones column
baked in (for the A row-sum) and is bf16.

Phase 1 (software-pipelined, skew 2: front(i) runs with tail(i-2)):
  front: DMA (prefetch +1, sync HWDGE queue); projKQ = planes^T @ [q|k]T
    (Q rows 0-63, K rows 64-127 of one PSUM bank via tile_position);
    tanhKQ (1 ACT op); logitsQT = blockdiag^T @ tanhQ (2 mm, head order
    0,2,4,6,1,3,5,7); expQT; logitsK via paired bd_eo (4 mm, base-64
    operands, natural head order); expK.
  tail:  gsum = ind^T @ expQT (2 mm); recipQS (DVE) -> bf16 cast;
    probsK = expK * recipK (group softmax16: DVE reduce + GPS multiply);
    bcast = indT^T @ recipQS (2 mm); stash probsQT bf16 (DVE);
    b_sum/A accumulate in 2 persistent PSUM banks via probsK^T @ [V|1].
  PSUM pool rotation order (gsum, proj, logitsQT, logitsK, recipQb) is
  chosen so no buffer reuse waits on a chain-end op.

Phase 2: E = b_sum/(A+eps) (ACT scale); out = stash^T @ E per (h, tile);
  bf16 output, row-interleaved DMA split across sync+scalar HWDGE queues
  (1KB packets spread over all 16 DMA engines). Host transposes
  (T,H,D)->(H,T,D) and casts back to f32.

Measured: ~232 us on-device (from 739 us baseline), rel err 9.6e-3.
"""

import itertools

import numpy as np
import ml_dtypes

import concourse.bacc as bacc
import concourse.mybir as mybir
import concourse.tile as tile

F32 = mybir.dt.float32
BF16 = mybir.dt.bfloat16
AF = mybir.ActivationFunctionType
MULT = mybir.AluOpType.mult

D_K, K_BITS, L_TAB, M_ENS = 64, 4, 8, 4
R = 1 << K_BITS          # 16
S = L_TAB * R            # 128
B, T, H = 2, 4096, 8
EPS = 1e-06
HD = H * D_K             # 512
TT = 128                 # T tile rows
NT = T // TT             # 32 tiles

# batched logitsQT column-block j holds head HEAD_AT[j]; POS inverts it
HEAD_AT = [0, 2, 4, 6, 1, 3, 5, 7]
POS = [HEAD_AT.index(h) for h in range(H)]


def _build_module():
    nc = bacc.Bacc("TRN2", target_bir_lowering=False, debug=False,
                   num_devices=8, enable_asserts=False)

    KT = nc.dram_tensor("KT", [128, 4, T], BF16, kind="ExternalInput")
    QT = nc.dram_tensor("QT", [128, 4, T], BF16, kind="ExternalInput")
    V = nc.dram_tensor("V", [T, H, 65], BF16, kind="ExternalInput")
    planes_both = nc.dram_tensor("planes_both", [128, 64], BF16, kind="ExternalInput")
    bd_eo2 = nc.dram_tensor("bd_eo2", [128, 256], BF16, kind="ExternalInput")
    blockdiag4 = nc.dram_tensor("blockdiag4", [128, 128], BF16, kind="ExternalInput")
    ind = nc.dram_tensor("ind", [128, 8], BF16, kind="ExternalInput")
    indT = nc.dram_tensor("indT", [8, 128], BF16, kind="ExternalInput")
    O = nc.dram_tensor("O", [T, HD], BF16, kind="ExternalOutput")

    with tile.TileContext(nc) as tc:
        with (
            tc.tile_pool(name="pconst", bufs=1) as pconst,
            tc.tile_pool(name="pstash", bufs=1) as pstash,
            tc.tile_pool(name="pin", bufs=5) as pin,
            tc.tile_pool(name="pmid", bufs=3) as pmid,
            tc.tile_pool(name="pout", bufs=6) as pout,
            tc.tile_pool(name="psmall", bufs=4) as psmall,
            tc.tile_pool(name="pacc", bufs=1, space="PSUM") as pacc,
            tc.tile_pool(name="pwork", bufs=3, space="PSUM") as pwork,
        ):
            # ---- first input tiles before consts (sync queue is idle)
            def issue_dma(ti):
                rows = slice(ti * TT, (ti + 1) * TT)
                kT_sb = pin.tile([128, 4, TT], BF16, tag="ktsb")
                nc.sync.dma_start(kT_sb[:], KT[:, :, rows])
                qT_sb = pin.tile([128, 4, TT], BF16, tag="qtsb")
                nc.sync.dma_start(qT_sb[:], QT[:, :, rows])
                tV = pin.tile([128, H, 65], BF16, tag="tv")
                nc.sync.dma_start(tV[:], V[rows, :, :])
                return {"kT": kT_sb, "qT": qT_sb, "tV": tV}

            stages = {}
            stages[0] = issue_dma(0)
            stages[1] = issue_dma(1)

            # ---- constants to SBUF
            planes_both_sb = pconst.tile([128, 64], BF16)
            nc.scalar.dma_start(planes_both_sb[:], planes_both[:])
            bd_eo2_sb = pconst.tile([128, 256], BF16)
            nc.scalar.dma_start(bd_eo2_sb[:], bd_eo2[:])
            blockdiag4_sb = pconst.tile([128, 128], BF16)
            nc.scalar.dma_start(blockdiag4_sb[:], blockdiag4[:])
            ind_sb = pconst.tile([128, 8], BF16)
            nc.scalar.dma_start(ind_sb[:], ind[:])
            indT_sb = pconst.tile([8, 128], BF16)
            nc.scalar.dma_start(indT_sb[:], indT[:])
            zrow = pconst.tile([1, 512], F32)
            nc.gpsimd.memset(zrow[:], 0.0)
            zcol = pconst.tile([1, 128], F32)
            nc.gpsimd.memset(zcol[:], 0.0)

            # ---- persistent PSUM accumulators: 4 heads per bank
            accA = pacc.tile([128, 4 * 65], F32)
            accB = pacc.tile([128, 4 * 65], F32)
            # zero-fill via a K=1 matmul of zeros: sets has_written for the
            # whole bank so every real b_sum matmul can accumulate
            # (start=False) in any order.
            nc.tensor.matmul(accA[:, 0:260], zcol[:], zrow[:, 0:260],
                             start=True, stop=False, skip_group_check=True)
            nc.tensor.matmul(accB[:, 0:260], zcol[:], zrow[:, 0:260],
                             start=True, stop=False, skip_group_check=True)

            # probsQ^T stash: (s, block j, tile, t) bf16
            stash = pstash.tile([128, H, NT, TT], BF16)

            # ================= phase 1 (software-pipelined) =================
            # front(i): DMA(prefetched), projKQ (Q rows 0-63, K rows 64-127),
            #   tanhKQ, logitsK (base-64 operands), logitsQT, expK, expQT
            # tail(i-1): gsum, recipQS, cast(GPS), bcast, stashTT,
            #   reduceK, recipK, probsK(GPS), b_sum
            def front(ti, S):
                projKQ = pwork.tile([128, 512], F32, tag="work")
                nc.tensor.matmul(projKQ[0:64, :], planes_both_sb[:],
                                 S["qT"][:].rearrange("q p t -> q (p t)"),
                                 start=True, stop=True)
                nc.tensor.matmul(projKQ[64:128, :], planes_both_sb[:],
                                 S["kT"][:].rearrange("q p t -> q (p t)"),
                                 start=True, stop=True)
                tanhKQ = pmid.tile([128, 512], BF16, tag="thkq")
                nc.scalar.activation(tanhKQ[:], projKQ[:], AF.Tanh)

                logitsQT = pwork.tile([128, 1024], F32, tag="work")
                nc.tensor.matmul(logitsQT[:, 0:512], blockdiag4_sb[0:32, :],
                                 tanhKQ[0:32, :], start=True, stop=True)
                nc.tensor.matmul(logitsQT[:, 512:1024],
                                 blockdiag4_sb[32:64, :],
                                 tanhKQ[32:64, :], start=True, stop=True)
                expQT = pmid.tile([128, 1024], BF16, tag="expq")
                nc.scalar.activation(expQT[:], logitsQT[:], AF.Exp)

                logitsK = pwork.tile([128, 1024], F32, tag="work")
                for p in range(4):
                    nc.tensor.matmul(
                        logitsK[:, p * 256:(p + 1) * 256],
                        tanhKQ[64:128, p * 128:(p + 1) * 128],
                        bd_eo2_sb[64:128, :], start=True, stop=True)
                expK = pmid.tile([128, 1024], BF16, tag="expk")
                nc.scalar.activation(expK[:], logitsK[:], AF.Exp)
                S["expK"] = expK
                S["expQT"] = expQT

            def tail_gsum(ti, S):
                gsumQ = pwork.tile([8, 1024], F32, tag="work")
                nc.tensor.matmul(gsumQ[:, 0:512], ind_sb[:],
                                 S["expQT"][:, 0:512], start=True, stop=True)
                nc.tensor.matmul(gsumQ[:, 512:1024], ind_sb[:],
                                 S["expQT"][:, 512:1024], start=True,
                                 stop=True)
                recipQS = pmid.tile([8, 1024], F32, tag="rqs")
                nc.vector.reciprocal_approx_fast(recipQS[:], gsumQ[:])
                recipQSb = pmid.tile([8, 1024], BF16, tag="rqsb")
                nc.vector.tensor_copy(recipQSb[:], recipQS[:])
                S["recipQSb"] = recipQSb

            def tail_probsk(ti, S):
                expK = S["expK"]
                denomK = pmid.tile([128, 64], F32, tag="dk")
                nc.vector.reduce_sum(
                    denomK[:],
                    expK[:].rearrange("p (g r) -> p g r", g=H * L_TAB),
                    axis=mybir.AxisListType.X)
                recipK = pmid.tile([128, 64], F32, tag="rk")
                nc.vector.reciprocal_approx_fast(recipK[:], denomK[:])
                probsK = pmid.tile([128, 1024], BF16, tag="pk")
                nc.gpsimd.tensor_tensor(
                    probsK[:].rearrange("p (h l r) -> p h l r", h=H, l=L_TAB),
                    expK[:].rearrange("p (h l r) -> p h l r", h=H, l=L_TAB),
                    recipK[:].rearrange("p (h l) -> p h l", h=H)
                        .broadcast_to((128, H, L_TAB, R)),
                    op=MULT)
                S["probsK"] = probsK

            def tail_rest(ti, S):
                recipQb = pwork.tile([128, 1024], F32, tag="work")
                nc.tensor.matmul(recipQb[:, 0:512], indT_sb[:],
                                 S["recipQSb"][:, 0:512], start=True,
                                 stop=True)
                nc.tensor.matmul(recipQb[:, 512:1024], indT_sb[:],
                                 S["recipQSb"][:, 512:1024], start=True,
                                 stop=True)
                nc.vector.tensor_tensor(
                    stash[:, :, ti, :],
                    S["expQT"][:].rearrange("p (h t) -> p h t", h=H),
                    recipQb[:].rearrange("p (h t) -> p h t", h=H),
                    op=MULT)

                probsK = S["probsK"]
                for h in range(H):
                    acc = accA if h < 4 else accB
                    off = (h % 4) * 65
                    nc.tensor.matmul(
                        acc[:, off:off + 65],
                        probsK[:, h * 128:(h + 1) * 128],
                        S["tV"][:, h, :],
                        start=False, stop=(ti == NT - 1 and h % 4 == 3),
                        skip_group_check=True)

            for ti in range(NT):
                if ti + 2 < NT:
                    stages[ti + 2] = issue_dma(ti + 2)
                if ti >= 2:
                    tail_gsum(ti - 2, stages[ti - 2])
                front(ti, stages[ti])
                if ti >= 2:
                    tail_probsk(ti - 2, stages[ti - 2])
                    tail_rest(ti - 2, stages[ti - 2])
                    del stages[ti - 2]
            for ti in (NT - 2, NT - 1):
                tail_probsk(ti, stages[ti])
                tail_gsum(ti, stages[ti])
                tail_rest(ti, stages[ti])

            # ================= E = b_sum / (A + eps) =================
            e_tiles = []
            for h in range(H):
                acc = accA if h < 4 else accB
                off = (h % 4) * 65
                aeps = psmall.tile([128, 1], F32, tag="ae")
                nc.vector.tensor_scalar_add(aeps[:], acc[:, off + 64:off + 65],
                                            EPS)
                recipA = psmall.tile([128, 1], F32, tag="ra")
                nc.vector.reciprocal_approx_fast(recipA[:], aeps[:])
                e_h = pconst.tile([128, 64], BF16, name=f"e_{h}")
                nc.scalar.activation(e_h[:], acc[:, off:off + 64], AF.Copy,
                                     scale=recipA[:])
                e_tiles.append(e_h)

            # ================= phase 2: out = probsQT.T @ E =================
            for ti in range(NT):
                out2 = pwork.tile([128, 512], F32, tag="work")
                for h in range(H):
                    nc.tensor.matmul(out2[:, h * 64:(h + 1) * 64],
                                     stash[:, POS[h], ti, :], e_tiles[h][:],
                                     start=True, stop=True)
                outT = pout.tile([128, 512], BF16, tag="ot")
                nc.vector.tensor_copy(outT[:], out2[:])
                base = ti * TT
                nc.sync.dma_start(O[base:base + TT:2, :], outT[0:TT:2, :])
                nc.scalar.dma_start(O[base + 1:base + TT:2, :],
                                    outT[1:TT:2, :])

    nc.finalize()
    return nc


def _protos() -> np.ndarray:
    corners = np.array(list(itertools.product([-1.0, 1.0], repeat=K_BITS)),
                       dtype=np.float32)
    return corners.T  # (K_BITS, R)


def _consts_for(planes_m: np.ndarray, scale: float) -> dict:
    protos_s = (_protos() / scale).astype(np.float32)  # (4, 16)
    blockdiag = np.zeros((32, 128), np.float32)
    for l in range(L_TAB):
        blockdiag[l * K_BITS:(l + 1) * K_BITS, l * R:(l + 1) * R] = protos_s
    planes_both = np.zeros((128, 64), np.float32)
    planes_both[0:64, 0:32] = planes_m
    planes_both[64:128, 32:64] = planes_m
    bd_eo2 = np.zeros((128, 256), np.float32)
    bd_eo2[64:96, 0:128] = blockdiag
    bd_eo2[96:128, 128:256] = blockdiag
    blockdiag4 = np.concatenate([blockdiag] * 4, axis=0)
    ind = np.zeros((128, 8), np.float32)
    for s in range(S):
        ind[s, s // R] = 1.0
    return {
        "planes_both": planes_both.astype(ml_dtypes.bfloat16),
        "bd_eo2": bd_eo2.astype(ml_dtypes.bfloat16),
        "blockdiag4": blockdiag4.astype(ml_dtypes.bfloat16),
        "ind": ind.astype(ml_dtypes.bfloat16),
        "indT": np.ascontiguousarray(ind.T).astype(ml_dtypes.bfloat16),
    }


_NC_CACHE = None


def _get_module():
    global _NC_CACHE
    if _NC_CACHE is None:
        _NC_CACHE = _build_module()
    return _NC_CACHE


def _v_ones(v):
    out = np.ones((T, H, 65), np.float32)
    out[:, :, 0:64] = v.reshape(T, H, 64)
    return out.astype(ml_dtypes.bfloat16)


def make_in_maps(Khf, Vhf, Qhf, planes_T, logit_temp):
    Khf = np.asarray(Khf, np.float32)
    Vhf = np.asarray(Vhf, np.float32)
    Qhf = np.asarray(Qhf, np.float32)
    planes_T = np.asarray(planes_T, np.float32)
    scale = float(np.clip(np.exp(float(np.asarray(logit_temp))), 0.01, 20.0))
    in_maps = []
    for c in range(8):
        m, b = c // 2, c % 2
        consts = _consts_for(planes_T[m], scale)
        def pre_t(x):
            # (T, H*D) -> (q=[d|d], p, T): q<64 is head 2p, q>=64 head 2p+1
            x3 = x.reshape(T, 4, 2, D_K)          # (t, p, r, d)
            return np.ascontiguousarray(
                x3.transpose(2, 3, 1, 0).reshape(128, 4, T)
            ).astype(ml_dtypes.bfloat16)
        in_maps.append({
            "KT": pre_t(Khf[m, b].reshape(T, HD)),
            "QT": pre_t(Qhf[m, b].reshape(T, HD)),
            "V": _v_ones(Vhf[m, b].reshape(T, HD)),
            **consts,
        })
    return in_maps


def assemble_output(results) -> np.ndarray:
    out = np.empty((M_ENS, B, H, T, D_K), np.float32)
    for c in range(8):
        out[c // 2, c % 2] = results[c]["O"].astype(np.float32).reshape(
            T, H, D_K).transpose(1, 0, 2)
    return out


def kernel(Khf, Vhf, Qhf, planes_T, logit_temp) -> np.ndarray:
    from concourse.bass_utils import run_bass_kernel_spmd
    nc = _get_module()
    in_maps = make_in_maps(Khf, Vhf, Qhf, planes_T, logit_temp)
    res = run_bass_kernel_spmd(nc, in_maps, list(range(8)))
    return assemble_output(res.results)


# revision 36
# speedup vs baseline: 1.0427x; 1.0005x over previous
"""BatchedACE (soft clustered linear attention) Trainium2 kernel.

Full inputs -> full output. Sharding: N = M*B*H batch axis across 8 cores;
core c handles (m, b) = (c//2, c%2), i.e. all 8 heads of one (ensemble,
batch) pair. Host pre-work per core: K/Q transposed to d-major (q=[d|d]
even/odd head pairs, p, T) and converted to bf16; V gets a ones column
baked in (for the A row-sum) and is bf16.

Phase 1 (software-pipelined, skew 2: front(i) runs with tail(i-2)):
  front: DMA (prefetch +1, sync HWDGE queue); projKQ = planes^T @ [q|k]T
    (Q rows 0-63, K rows 64-127 of one PSUM bank via tile_position);
    tanhKQ (1 ACT op); logitsQT = blockdiag^T @ tanhQ (2 mm, head order
    0,2,4,6,1,3,5,7); expQT; logitsK via paired bd_eo (4 mm, base-64
    operands, natural head order); expK.
  tail:  gsum = ind^T @ expQT (2 mm); recipQS (DVE) -> bf16 cast;
    probsK = expK * recipK (group softmax16: DVE reduce + GPS multiply);
    bcast = indT^T @ recipQS (2 mm); stash probsQT bf16 (DVE);
    b_sum/A accumulate in 2 persistent PSUM banks via probsK^T @ [V|1].
  PSUM pool rotation order (gsum, proj, logitsQT, logitsK, recipQb) is
  chosen so no buffer reuse waits on a chain-end op.

Phase 2: E = b_sum/(A+eps) (ACT scale); out = stash^T @ E per (h, tile);
  bf16 output, row-interleaved DMA split across sync+scalar HWDGE queues
  (1KB packets spread over all 16 DMA engines). Host transposes
  (T,H,D)->(H,T,D) and casts back to f32.

Measured: ~232 us on-device (from 739 us baseline), rel err 9.6e-3.
"""

import itertools

import numpy as np
import ml_dtypes

import concourse.bacc as bacc
import concourse.mybir as mybir
import concourse.tile as tile

F32 = mybir.dt.float32
BF16 = mybir.dt.bfloat16
AF = mybir.ActivationFunctionType
MULT = mybir.AluOpType.mult

D_K, K_BITS, L_TAB, M_ENS = 64, 4, 8, 4
R = 1 << K_BITS          # 16
S = L_TAB * R            # 128
B, T, H = 2, 4096, 8
EPS = 1e-06
HD = H * D_K             # 512
TT = 128                 # T tile rows
NT = T // TT             # 32 tiles

# batched logitsQT column-block j holds head HEAD_AT[j]; POS inverts it
HEAD_AT = [0, 2, 4, 6, 1, 3, 5, 7]
POS = [HEAD_AT.index(h) for h in range(H)]


def _build_module():
    nc = bacc.Bacc("TRN2", target_bir_lowering=False, debug=False,
                   num_devices=8, enable_asserts=False)

    KT = nc.dram_tensor("KT", [128, 4, T], BF16, kind="ExternalInput")
    QT = nc.dram_tensor("QT", [128, 4, T], BF16, kind="ExternalInput")
    V = nc.dram_tensor("V", [T, H, 65], BF16, kind="ExternalInput")
    planes_both = nc.dram_tensor("planes_both", [128, 64], BF16, kind="ExternalInput")
    bd_eo2 = nc.dram_tensor("bd_eo2", [128, 256], BF16, kind="ExternalInput")
    blockdiag4 = nc.dram_tensor("blockdiag4", [128, 128], BF16, kind="ExternalInput")
    ind = nc.dram_tensor("ind", [128, 8], BF16, kind="ExternalInput")
    indT = nc.dram_tensor("indT", [8, 128], BF16, kind="ExternalInput")
    O = nc.dram_tensor("O", [T, HD], BF16, kind="ExternalOutput")

    with tile.TileContext(nc) as tc:
        with (
            tc.tile_pool(name="pconst", bufs=1) as pconst,
            tc.tile_pool(name="pstash", bufs=1) as pstash,
            tc.tile_pool(name="pin", bufs=5) as pin,
            tc.tile_pool(name="pmid", bufs=4) as pmid,
            tc.tile_pool(name="pout", bufs=6) as pout,
            tc.tile_pool(name="psmall", bufs=4) as psmall,
            tc.tile_pool(name="pacc", bufs=1, space="PSUM") as pacc,
            tc.tile_pool(name="pwork", bufs=3, space="PSUM") as pwork,
        ):
            # ---- first input tiles before consts (sync queue is idle)
            def issue_dma(ti):
                rows = slice(ti * TT, (ti + 1) * TT)
                kT_sb = pin.tile([128, 4, TT], BF16, tag="ktsb")
                nc.sync.dma_start(kT_sb[:], KT[:, :, rows])
                qT_sb = pin.tile([128, 4, TT], BF16, tag="qtsb")
                nc.sync.dma_start(qT_sb[:], QT[:, :, rows])
                tV = pin.tile([128, H, 65], BF16, tag="tv")
                nc.sync.dma_start(tV[:], V[rows, :, :])
                return {"kT": kT_sb, "qT": qT_sb, "tV": tV}

            stages = {}
            stages[0] = issue_dma(0)
            stages[1] = issue_dma(1)

            # ---- constants to SBUF
            planes_both_sb = pconst.tile([128, 64], BF16)
            nc.scalar.dma_start(planes_both_sb[:], planes_both[:])
            bd_eo2_sb = pconst.tile([128, 256], BF16)
            nc.scalar.dma_start(bd_eo2_sb[:], bd_eo2[:])
            blockdiag4_sb = pconst.tile([128, 128], BF16)
            nc.scalar.dma_start(blockdiag4_sb[:], blockdiag4[:])
            ind_sb = pconst.tile([128, 8], BF16)
            nc.scalar.dma_start(ind_sb[:], ind[:])
            indT_sb = pconst.tile([8, 128], BF16)
            nc.scalar.dma_start(indT_sb[:], indT[:])
            zrow = pconst.tile([1, 512], F32)
            nc.gpsimd.memset(zrow[:], 0.0)
            zcol = pconst.tile([1, 128], F32)
            nc.gpsimd.memset(zcol[:], 0.0)

            # ---- persistent PSUM accumulators: 4 heads per bank
            accA = pacc.tile([128, 4 * 65], F32)
            accB = pacc.tile([128, 4 * 65], F32)
            # zero-fill via a K=1 matmul of zeros: sets has_written for the
            # whole bank so every real b_sum matmul can accumulate
            # (start=False) in any order.
            nc.tensor.matmul(accA[:, 0:260], zcol[:], zrow[:, 0:260],
                             start=True, stop=False, skip_group_check=True)
            nc.tensor.matmul(accB[:, 0:260], zcol[:], zrow[:, 0:260],
                             start=True, stop=False, skip_group_check=True)

            # probsQ^T stash: (s, block j, tile, t) bf16
            stash = pstash.tile([128, H, NT, TT], BF16)

            # ================= phase 1 (software-pipelined) =================
            # front(i): DMA(prefetched), projKQ (Q rows 0-63, K rows 64-127),
            #   tanhKQ, logitsK (base-64 operands), logitsQT, expK, expQT
            # tail(i-1): gsum, recipQS, cast(GPS), bcast, stashTT,
            #   reduceK, recipK, probsK(GPS), b_sum
            def front(ti, S):
                projKQ = pwork.tile([128, 512], F32, tag="work")
                nc.tensor.matmul(projKQ[0:64, :], planes_both_sb[:],
                                 S["qT"][:].rearrange("q p t -> q (p t)"),
                                 start=True, stop=True)
                nc.tensor.matmul(projKQ[64:128, :], planes_both_sb[:],
                                 S["kT"][:].rearrange("q p t -> q (p t)"),
                                 start=True, stop=True)
                tanhKQ = pmid.tile([128, 512], BF16, tag="thkq")
                nc.scalar.activation(tanhKQ[:], projKQ[:], AF.Tanh)

                logitsQT = pwork.tile([128, 1024], F32, tag="work")
                nc.tensor.matmul(logitsQT[:, 0:512], blockdiag4_sb[0:32, :],
                                 tanhKQ[0:32, :], start=True, stop=True)
                nc.tensor.matmul(logitsQT[:, 512:1024],
                                 blockdiag4_sb[32:64, :],
                                 tanhKQ[32:64, :], start=True, stop=True)
                expQT = pmid.tile([128, 1024], BF16, tag="expq")
                nc.scalar.activation(expQT[:], logitsQT[:], AF.Exp)

                logitsK = pwork.tile([128, 1024], F32, tag="work")
                for p in range(4):
                    nc.tensor.matmul(
                        logitsK[:, p * 256:(p + 1) * 256],
                        tanhKQ[64:128, p * 128:(p + 1) * 128],
                        bd_eo2_sb[64:128, :], start=True, stop=True)
                expK = pmid.tile([128, 1024], BF16, tag="expk")
                nc.scalar.activation(expK[:], logitsK[:], AF.Exp)
                S["expK"] = expK
                S["expQT"] = expQT

            def tail_gsum(ti, S):
                gsumQ = pwork.tile([8, 1024], F32, tag="work")
                nc.tensor.matmul(gsumQ[:, 0:512], ind_sb[:],
                                 S["expQT"][:, 0:512], start=True, stop=True)
                nc.tensor.matmul(gsumQ[:, 512:1024], ind_sb[:],
                                 S["expQT"][:, 512:1024], start=True,
                                 stop=True)
                recipQS = pmid.tile([8, 1024], F32, tag="rqs")
                nc.vector.reciprocal_approx_fast(recipQS[:], gsumQ[:])
                recipQSb = pmid.tile([8, 1024], BF16, tag="rqsb")
                nc.vector.tensor_copy(recipQSb[:], recipQS[:])
                S["recipQSb"] = recipQSb

            def tail_probsk(ti, S):
                expK = S["expK"]
                denomK = pmid.tile([128, 64], F32, tag="dk")
                nc.vector.reduce_sum(
                    denomK[:],
                    expK[:].rearrange("p (g r) -> p g r", g=H * L_TAB),
                    axis=mybir.AxisListType.X)
                recipK = pmid.tile([128, 64], F32, tag="rk")
                nc.vector.reciprocal_approx_fast(recipK[:], denomK[:])
                probsK = pmid.tile([128, 1024], BF16, tag="pk")
                nc.gpsimd.tensor_tensor(
                    probsK[:].rearrange("p (h l r) -> p h l r", h=H, l=L_TAB),
                    expK[:].rearrange("p (h l r) -> p h l r", h=H, l=L_TAB),
                    recipK[:].rearrange("p (h l) -> p h l", h=H)
                        .broadcast_to((128, H, L_TAB, R)),
                    op=MULT)
                S["probsK"] = probsK

            def tail_rest(ti, S):
                recipQb = pwork.tile([128, 1024], F32, tag="work")
                nc.tensor.matmul(recipQb[:, 0:512], indT_sb[:],
                                 S["recipQSb"][:, 0:512], start=True,
                                 stop=True)
                nc.tensor.matmul(recipQb[:, 512:1024], indT_sb[:],
                                 S["recipQSb"][:, 512:1024], start=True,
                                 stop=True)
                nc.vector.tensor_tensor(
                    stash[:, :, ti, :],
                    S["expQT"][:].rearrange("p (h t) -> p h t", h=H),
                    recipQb[:].rearrange("p (h t) -> p h t", h=H),
                    op=MULT)

                probsK = S["probsK"]
                for h in range(H):
                    acc = accA if h < 4 else accB
                    off = (h % 4) * 65
                    nc.tensor.matmul(
                        acc[:, off:off + 65],
                        probsK[:, h * 128:(h + 1) * 128],
                        S["tV"][:, h, :],
                        start=False, stop=(ti == NT - 1 and h % 4 == 3),
                        skip_group_check=True)

            for ti in range(NT):
                if ti + 2 < NT:
                    stages[ti + 2] = issue_dma(ti + 2)
                if ti >= 2:
                    tail_gsum(ti - 2, stages[ti - 2])
                front(ti, stages[ti])
                if ti >= 2:
                    tail_probsk(ti - 2, stages[ti - 2])
                    tail_rest(ti - 2, stages[ti - 2])
                    del stages[ti - 2]
            for ti in (NT - 2, NT - 1):
                tail_probsk(ti, stages[ti])
                tail_gsum(ti, stages[ti])
                tail_rest(ti, stages[ti])

            # ================= E = b_sum / (A + eps) =================
            e_tiles = []
            for h in range(H):
                acc = accA if h < 4 else accB
                off = (h % 4) * 65
                aeps = psmall.tile([128, 1], F32, tag="ae")
                nc.vector.tensor_scalar_add(aeps[:], acc[:, off + 64:off + 65],
                                            EPS)
                recipA = psmall.tile([128, 1], F32, tag="ra")
                nc.vector.reciprocal_approx_fast(recipA[:], aeps[:])
                e_h = pconst.tile([128, 64], BF16, name=f"e_{h}")
                nc.scalar.activation(e_h[:], acc[:, off:off + 64], AF.Copy,
                                     scale=recipA[:])
                e_tiles.append(e_h)

            # ================= phase 2: out = probsQT.T @ E =================
            for ti in range(NT):
                out2 = pwork.tile([128, 512], F32, tag="work")
                for h in range(H):
                    nc.tensor.matmul(out2[:, h * 64:(h + 1) * 64],
                                     stash[:, POS[h], ti, :], e_tiles[h][:],
                                     start=True, stop=True)
                outT = pout.tile([128, 512], BF16, tag="ot")
                nc.vector.tensor_copy(outT[:], out2[:])
                base = ti * TT
                nc.sync.dma_start(O[base:base + TT:2, :], outT[0:TT:2, :])
                nc.scalar.dma_start(O[base + 1:base + TT:2, :],
                                    outT[1:TT:2, :])

    nc.finalize()
    return nc


def _protos() -> np.ndarray:
    corners = np.array(list(itertools.product([-1.0, 1.0], repeat=K_BITS)),
                       dtype=np.float32)
    return corners.T  # (K_BITS, R)


def _consts_for(planes_m: np.ndarray, scale: float) -> dict:
    protos_s = (_protos() / scale).astype(np.float32)  # (4, 16)
    blockdiag = np.zeros((32, 128), np.float32)
    for l in range(L_TAB):
        blockdiag[l * K_BITS:(l + 1) * K_BITS, l * R:(l + 1) * R] = protos_s
    planes_both = np.zeros((128, 64), np.float32)
    planes_both[0:64, 0:32] = planes_m
    planes_both[64:128, 32:64] = planes_m
    bd_eo2 = np.zeros((128, 256), np.float32)
    bd_eo2[64:96, 0:128] = blockdiag
    bd_eo2[96:128, 128:256] = blockdiag
    blockdiag4 = np.concatenate([blockdiag] * 4, axis=0)
    ind = np.zeros((128, 8), np.float32)
    for s in range(S):
        ind[s, s // R] = 1.0
    return {
        "planes_both": planes_both.astype(ml_dtypes.bfloat16),
        "bd_eo2": bd_eo2.astype(ml_dtypes.bfloat16),
        "blockdiag4": blockdiag4.astype(ml_dtypes.bfloat16),
        "ind": ind.astype(ml_dtypes.bfloat16),
        "indT": np.ascontiguousarray(ind.T).astype(ml_dtypes.bfloat16),
    }


_NC_CACHE = None


def _get_module():
    global _NC_CACHE
    if _NC_CACHE is None:
        _NC_CACHE = _build_module()
    return _NC_CACHE


def _v_ones(v):
    out = np.ones((T, H, 65), np.float32)
    out[:, :, 0:64] = v.reshape(T, H, 64)
    return out.astype(ml_dtypes.bfloat16)


def make_in_maps(Khf, Vhf, Qhf, planes_T, logit_temp):
    Khf = np.asarray(Khf, np.float32)
    Vhf = np.asarray(Vhf, np.float32)
    Qhf = np.asarray(Qhf, np.float32)
    planes_T = np.asarray(planes_T, np.float32)
    scale = float(np.clip(np.exp(float(np.asarray(logit_temp))), 0.01, 20.0))
    in_maps = []
    for c in range(8):
        m, b = c // 2, c % 2
        consts = _consts_for(planes_T[m], scale)
        def pre_t(x):
            # (T, H*D) -> (q=[d|d], p, T): q<64 is head 2p, q>=64 head 2p+1
            x3 = x.reshape(T, 4, 2, D_K)          # (t, p, r, d)
            return np.ascontiguousarray(
                x3.transpose(2, 3, 1, 0).reshape(128, 4, T)
            ).astype(ml_dtypes.bfloat16)
        in_maps.append({
            "KT": pre_t(Khf[m, b].reshape(T, HD)),
            "QT": pre_t(Qhf[m, b].reshape(T, HD)),
            "V": _v_ones(Vhf[m, b].reshape(T, HD)),
            **consts,
        })
    return in_maps


def assemble_output(results) -> np.ndarray:
    out = np.empty((M_ENS, B, H, T, D_K), np.float32)
    for c in range(8):
        out[c // 2, c % 2] = results[c]["O"].astype(np.float32).reshape(
            T, H, D_K).transpose(1, 0, 2)
    return out


def kernel(Khf, Vhf, Qhf, planes_T, logit_temp) -> np.ndarray:
    from concourse.bass_utils import run_bass_kernel_spmd
    nc = _get_module()
    in_maps = make_in_maps(Khf, Vhf, Qhf, planes_T, logit_temp)
    res = run_bass_kernel_spmd(nc, in_maps, list(range(8)))
    return assemble_output(res.results)


# revision 37
# speedup vs baseline: 1.0466x; 1.0037x over previous
"""BatchedACE (soft clustered linear attention) Trainium2 kernel.

Full inputs -> full output. Sharding: N = M*B*H batch axis across 8 cores;
core c handles (m, b) = (c//2, c%2), i.e. all 8 heads of one (ensemble,
batch) pair. Host pre-work per core: K/Q transposed to d-major (q=[d|d]
even/odd head pairs, p, T) and converted to bf16; V gets a ones column
baked in (for the A row-sum) and is bf16.

Phase 1 (software-pipelined, skew 2: front(i) runs with tail(i-2)):
  front: DMA (prefetch +1, sync HWDGE queue); projKQ = planes^T @ [q|k]T
    (Q rows 0-63, K rows 64-127 of one PSUM bank via tile_position);
    tanhKQ (1 ACT op); logitsQT = blockdiag^T @ tanhQ (2 mm, head order
    0,2,4,6,1,3,5,7); expQT; logitsK via paired bd_eo (4 mm, base-64
    operands, natural head order); expK.
  tail:  gsum = ind^T @ expQT (2 mm); recipQS (DVE) -> bf16 cast;
    probsK = expK * recipK (group softmax16: DVE reduce + GPS multiply);
    bcast = indT^T @ recipQS (2 mm); stash probsQT bf16 (DVE);
    b_sum/A accumulate in 2 persistent PSUM banks via probsK^T @ [V|1].
  PSUM pool rotation order (gsum, proj, logitsQT, logitsK, recipQb) is
  chosen so no buffer reuse waits on a chain-end op.

Phase 2: E = b_sum/(A+eps) (ACT scale); out = stash^T @ E per (h, tile);
  bf16 output, row-interleaved DMA split across sync+scalar HWDGE queues
  (1KB packets spread over all 16 DMA engines). Host transposes
  (T,H,D)->(H,T,D) and casts back to f32.

Measured: ~232 us on-device (from 739 us baseline), rel err 9.6e-3.
"""

import itertools

import numpy as np
import ml_dtypes

import concourse.bacc as bacc
import concourse.mybir as mybir
import concourse.tile as tile

F32 = mybir.dt.float32
BF16 = mybir.dt.bfloat16
AF = mybir.ActivationFunctionType
MULT = mybir.AluOpType.mult

D_K, K_BITS, L_TAB, M_ENS = 64, 4, 8, 4
R = 1 << K_BITS          # 16
S = L_TAB * R            # 128
B, T, H = 2, 4096, 8
EPS = 1e-06
HD = H * D_K             # 512
TT = 128                 # T tile rows
NT = T // TT             # 32 tiles

# batched logitsQT column-block j holds head HEAD_AT[j]; POS inverts it
HEAD_AT = [0, 2, 4, 6, 1, 3, 5, 7]
POS = [HEAD_AT.index(h) for h in range(H)]


def _build_module():
    nc = bacc.Bacc("TRN2", target_bir_lowering=False, debug=False,
                   num_devices=8, enable_asserts=False)

    KT = nc.dram_tensor("KT", [128, 4, T], BF16, kind="ExternalInput")
    QT = nc.dram_tensor("QT", [128, 4, T], BF16, kind="ExternalInput")
    V = nc.dram_tensor("V", [T, H, 65], BF16, kind="ExternalInput")
    planes_both = nc.dram_tensor("planes_both", [128, 64], BF16, kind="ExternalInput")
    bd_eo2 = nc.dram_tensor("bd_eo2", [128, 256], BF16, kind="ExternalInput")
    blockdiag4 = nc.dram_tensor("blockdiag4", [128, 128], BF16, kind="ExternalInput")
    ind = nc.dram_tensor("ind", [128, 8], BF16, kind="ExternalInput")
    indT = nc.dram_tensor("indT", [8, 128], BF16, kind="ExternalInput")
    O = nc.dram_tensor("O", [T, HD], BF16, kind="ExternalOutput")

    with tile.TileContext(nc) as tc:
        with (
            tc.tile_pool(name="pconst", bufs=1) as pconst,
            tc.tile_pool(name="pstash", bufs=1) as pstash,
            tc.tile_pool(name="pin", bufs=5) as pin,
            tc.tile_pool(name="pmid", bufs=4) as pmid,
            tc.tile_pool(name="pout", bufs=8) as pout,
            tc.tile_pool(name="psmall", bufs=4) as psmall,
            tc.tile_pool(name="pacc", bufs=1, space="PSUM") as pacc,
            tc.tile_pool(name="pwork", bufs=3, space="PSUM") as pwork,
        ):
            # ---- first input tiles before consts (sync queue is idle)
            def issue_dma(ti):
                rows = slice(ti * TT, (ti + 1) * TT)
                kT_sb = pin.tile([128, 4, TT], BF16, tag="ktsb")
                nc.sync.dma_start(kT_sb[:], KT[:, :, rows])
                qT_sb = pin.tile([128, 4, TT], BF16, tag="qtsb")
                nc.sync.dma_start(qT_sb[:], QT[:, :, rows])
                tV = pin.tile([128, H, 65], BF16, tag="tv")
                nc.sync.dma_start(tV[:], V[rows, :, :])
                return {"kT": kT_sb, "qT": qT_sb, "tV": tV}

            stages = {}
            stages[0] = issue_dma(0)
            stages[1] = issue_dma(1)

            # ---- constants to SBUF
            planes_both_sb = pconst.tile([128, 64], BF16)
            nc.scalar.dma_start(planes_both_sb[:], planes_both[:])
            bd_eo2_sb = pconst.tile([128, 256], BF16)
            nc.scalar.dma_start(bd_eo2_sb[:], bd_eo2[:])
            blockdiag4_sb = pconst.tile([128, 128], BF16)
            nc.scalar.dma_start(blockdiag4_sb[:], blockdiag4[:])
            ind_sb = pconst.tile([128, 8], BF16)
            nc.scalar.dma_start(ind_sb[:], ind[:])
            indT_sb = pconst.tile([8, 128], BF16)
            nc.scalar.dma_start(indT_sb[:], indT[:])
            zrow = pconst.tile([1, 512], F32)
            nc.gpsimd.memset(zrow[:], 0.0)
            zcol = pconst.tile([1, 128], F32)
            nc.gpsimd.memset(zcol[:], 0.0)

            # ---- persistent PSUM accumulators: 4 heads per bank
            accA = pacc.tile([128, 4 * 65], F32)
            accB = pacc.tile([128, 4 * 65], F32)
            # zero-fill via a K=1 matmul of zeros: sets has_written for the
            # whole bank so every real b_sum matmul can accumulate
            # (start=False) in any order.
            nc.tensor.matmul(accA[:, 0:260], zcol[:], zrow[:, 0:260],
                             start=True, stop=False, skip_group_check=True)
            nc.tensor.matmul(accB[:, 0:260], zcol[:], zrow[:, 0:260],
                             start=True, stop=False, skip_group_check=True)

            # probsQ^T stash: (s, block j, tile, t) bf16
            stash = pstash.tile([128, H, NT, TT], BF16)

            # ================= phase 1 (software-pipelined) =================
            # front(i): DMA(prefetched), projKQ (Q rows 0-63, K rows 64-127),
            #   tanhKQ, logitsK (base-64 operands), logitsQT, expK, expQT
            # tail(i-1): gsum, recipQS, cast(GPS), bcast, stashTT,
            #   reduceK, recipK, probsK(GPS), b_sum
            def front(ti, S):
                projKQ = pwork.tile([128, 512], F32, tag="work")
                nc.tensor.matmul(projKQ[0:64, :], planes_both_sb[:],
                                 S["qT"][:].rearrange("q p t -> q (p t)"),
                                 start=True, stop=True)
                nc.tensor.matmul(projKQ[64:128, :], planes_both_sb[:],
                                 S["kT"][:].rearrange("q p t -> q (p t)"),
                                 start=True, stop=True)
                tanhKQ = pmid.tile([128, 512], BF16, tag="thkq")
                nc.scalar.activation(tanhKQ[:], projKQ[:], AF.Tanh)

                logitsQT = pwork.tile([128, 1024], F32, tag="work")
                nc.tensor.matmul(logitsQT[:, 0:512], blockdiag4_sb[0:32, :],
                                 tanhKQ[0:32, :], start=True, stop=True)
                nc.tensor.matmul(logitsQT[:, 512:1024],
                                 blockdiag4_sb[32:64, :],
                                 tanhKQ[32:64, :], start=True, stop=True)
                expQT = pmid.tile([128, 1024], BF16, tag="expq")
                nc.scalar.activation(expQT[:], logitsQT[:], AF.Exp)

                logitsK = pwork.tile([128, 1024], F32, tag="work")
                for p in range(4):
                    nc.tensor.matmul(
                        logitsK[:, p * 256:(p + 1) * 256],
                        tanhKQ[64:128, p * 128:(p + 1) * 128],
                        bd_eo2_sb[64:128, :], start=True, stop=True)
                expK = pmid.tile([128, 1024], BF16, tag="expk")
                nc.scalar.activation(expK[:], logitsK[:], AF.Exp)
                S["expK"] = expK
                S["expQT"] = expQT

            def tail_gsum(ti, S):
                gsumQ = pwork.tile([8, 1024], F32, tag="work")
                nc.tensor.matmul(gsumQ[:, 0:512], ind_sb[:],
                                 S["expQT"][:, 0:512], start=True, stop=True)
                nc.tensor.matmul(gsumQ[:, 512:1024], ind_sb[:],
                                 S["expQT"][:, 512:1024], start=True,
                                 stop=True)
                recipQS = pmid.tile([8, 1024], F32, tag="rqs")
                nc.vector.reciprocal_approx_fast(recipQS[:], gsumQ[:])
                recipQSb = pmid.tile([8, 1024], BF16, tag="rqsb")
                nc.vector.tensor_copy(recipQSb[:], recipQS[:])
                S["recipQSb"] = recipQSb

            def tail_probsk(ti, S):
                expK = S["expK"]
                denomK = pmid.tile([128, 64], F32, tag="dk")
                nc.vector.reduce_sum(
                    denomK[:],
                    expK[:].rearrange("p (g r) -> p g r", g=H * L_TAB),
                    axis=mybir.AxisListType.X)
                recipK = pmid.tile([128, 64], F32, tag="rk")
                nc.vector.reciprocal_approx_fast(recipK[:], denomK[:])
                probsK = pmid.tile([128, 1024], BF16, tag="pk")
                nc.gpsimd.tensor_tensor(
                    probsK[:].rearrange("p (h l r) -> p h l r", h=H, l=L_TAB),
                    expK[:].rearrange("p (h l r) -> p h l r", h=H, l=L_TAB),
                    recipK[:].rearrange("p (h l) -> p h l", h=H)
                        .broadcast_to((128, H, L_TAB, R)),
                    op=MULT)
                S["probsK"] = probsK

            def tail_rest(ti, S):
                recipQb = pwork.tile([128, 1024], F32, tag="work")
                nc.tensor.matmul(recipQb[:, 0:512], indT_sb[:],
                                 S["recipQSb"][:, 0:512], start=True,
                                 stop=True)
                nc.tensor.matmul(recipQb[:, 512:1024], indT_sb[:],
                                 S["recipQSb"][:, 512:1024], start=True,
                                 stop=True)
                nc.vector.tensor_tensor(
                    stash[:, :, ti, :],
                    S["expQT"][:].rearrange("p (h t) -> p h t", h=H),
                    recipQb[:].rearrange("p (h t) -> p h t", h=H),
                    op=MULT)

                probsK = S["probsK"]
                for h in range(H):
                    acc = accA if h < 4 else accB
                    off = (h % 4) * 65
                    nc.tensor.matmul(
                        acc[:, off:off + 65],
                        probsK[:, h * 128:(h + 1) * 128],
                        S["tV"][:, h, :],
                        start=False, stop=(ti == NT - 1 and h % 4 == 3),
                        skip_group_check=True)

            for ti in range(NT):
                if ti + 2 < NT:
                    stages[ti + 2] = issue_dma(ti + 2)
                if ti >= 2:
                    tail_gsum(ti - 2, stages[ti - 2])
                front(ti, stages[ti])
                if ti >= 2:
                    tail_probsk(ti - 2, stages[ti - 2])
                    tail_rest(ti - 2, stages[ti - 2])
                    del stages[ti - 2]
            for ti in (NT - 2, NT - 1):
                tail_probsk(ti, stages[ti])
                tail_gsum(ti, stages[ti])
                tail_rest(ti, stages[ti])

            # ================= E = b_sum / (A + eps) =================
            e_tiles = []
            for h in range(H):
                acc = accA if h < 4 else accB
                off = (h % 4) * 65
                aeps = psmall.tile([128, 1], F32, tag="ae")
                nc.vector.tensor_scalar_add(aeps[:], acc[:, off + 64:off + 65],
                                            EPS)
                recipA = psmall.tile([128, 1], F32, tag="ra")
                nc.vector.reciprocal_approx_fast(recipA[:], aeps[:])
                e_h = pconst.tile([128, 64], BF16, name=f"e_{h}")
                nc.scalar.activation(e_h[:], acc[:, off:off + 64], AF.Copy,
                                     scale=recipA[:])
                e_tiles.append(e_h)

            # ================= phase 2: out = probsQT.T @ E =================
            for ti in range(NT):
                out2 = pwork.tile([128, 512], F32, tag="work")
                for h in range(H):
                    nc.tensor.matmul(out2[:, h * 64:(h + 1) * 64],
                                     stash[:, POS[h], ti, :], e_tiles[h][:],
                                     start=True, stop=True)
                outT = pout.tile([128, 512], BF16, tag="ot")
                nc.vector.tensor_copy(outT[:], out2[:])
                base = ti * TT
                nc.sync.dma_start(O[base:base + TT:2, :], outT[0:TT:2, :])
                nc.scalar.dma_start(O[base + 1:base + TT:2, :],
                                    outT[1:TT:2, :])

    nc.finalize()
    return nc


def _protos() -> np.ndarray:
    corners = np.array(list(itertools.product([-1.0, 1.0], repeat=K_BITS)),
                       dtype=np.float32)
    return corners.T  # (K_BITS, R)


def _consts_for(planes_m: np.ndarray, scale: float) -> dict:
    protos_s = (_protos() / scale).astype(np.float32)  # (4, 16)
    blockdiag = np.zeros((32, 128), np.float32)
    for l in range(L_TAB):
        blockdiag[l * K_BITS:(l + 1) * K_BITS, l * R:(l + 1) * R] = protos_s
    planes_both = np.zeros((128, 64), np.float32)
    planes_both[0:64, 0:32] = planes_m
    planes_both[64:128, 32:64] = planes_m
    bd_eo2 = np.zeros((128, 256), np.float32)
    bd_eo2[64:96, 0:128] = blockdiag
    bd_eo2[96:128, 128:256] = blockdiag
    blockdiag4 = np.concatenate([blockdiag] * 4, axis=0)
    ind = np.zeros((128, 8), np.float32)
    for s in range(S):
        ind[s, s // R] = 1.0
    return {
        "planes_both": planes_both.astype(ml_dtypes.bfloat16),
        "bd_eo2": bd_eo2.astype(ml_dtypes.bfloat16),
        "blockdiag4": blockdiag4.astype(ml_dtypes.bfloat16),
        "ind": ind.astype(ml_dtypes.bfloat16),
        "indT": np.ascontiguousarray(ind.T).astype(ml_dtypes.bfloat16),
    }


_NC_CACHE = None


def _get_module():
    global _NC_CACHE
    if _NC_CACHE is None:
        _NC_CACHE = _build_module()
    return _NC_CACHE


def _v_ones(v):
    out = np.ones((T, H, 65), np.float32)
    out[:, :, 0:64] = v.reshape(T, H, 64)
    return out.astype(ml_dtypes.bfloat16)


def make_in_maps(Khf, Vhf, Qhf, planes_T, logit_temp):
    Khf = np.asarray(Khf, np.float32)
    Vhf = np.asarray(Vhf, np.float32)
    Qhf = np.asarray(Qhf, np.float32)
    planes_T = np.asarray(planes_T, np.float32)
    scale = float(np.clip(np.exp(float(np.asarray(logit_temp))), 0.01, 20.0))
    in_maps = []
    for c in range(8):
        m, b = c // 2, c % 2
        consts = _consts_for(planes_T[m], scale)
        def pre_t(x):
            # (T, H*D) -> (q=[d|d], p, T): q<64 is head 2p, q>=64 head 2p+1
            x3 = x.reshape(T, 4, 2, D_K)          # (t, p, r, d)
            return np.ascontiguousarray(
                x3.transpose(2, 3, 1, 0).reshape(128, 4, T)
            ).astype(ml_dtypes.bfloat16)
        in_maps.append({
            "KT": pre_t(Khf[m, b].reshape(T, HD)),
            "QT": pre_t(Qhf[m, b].reshape(T, HD)),
            "V": _v_ones(Vhf[m, b].reshape(T, HD)),
            **consts,
        })
    return in_maps


def assemble_output(results) -> np.ndarray:
    out = np.empty((M_ENS, B, H, T, D_K), np.float32)
    for c in range(8):
        out[c // 2, c % 2] = results[c]["O"].astype(np.float32).reshape(
            T, H, D_K).transpose(1, 0, 2)
    return out


def kernel(Khf, Vhf, Qhf, planes_T, logit_temp) -> np.ndarray:
    from concourse.bass_utils import run_bass_kernel_spmd
    nc = _get_module()
    in_maps = make_in_maps(Khf, Vhf, Qhf, planes_T, logit_temp)
    res = run_bass_kernel_spmd(nc, in_maps, list(range(8)))
    return assemble_output(res.results)
